# revision 1
# baseline (speedup 1.0000x reference)
"""Mixed causal attention (B=8,L=1024,D=1024,H=16,NS=8) on 8 TRN2 cores.

Sharding: data-parallel over batch (core b owns batch b) for projections,
attention, out-proj.  The per-position (ns) projection weights are sharded
by position: core c computes q/k/v for position 1016+c for ALL batches
(reads only Wq_ns[c],Wk_ns[c],Wv_ns[c]), then an AllGather distributes the
3x[8,1024] results; each core extracts its batch's 8 tail rows with a
one-hot selection matmul (the program is SPMD-identical, so per-core row
selection is driven by a per-core input, not program structure).
"""

import sys
import os
from contextlib import ExitStack

import numpy as np

sys.path.insert(0, "/opt/trn_rl_repo")

import ml_dtypes  # noqa: E402
import concourse.bass as bass  # noqa: E402
import concourse.tile as tile  # noqa: E402
from concourse import bacc, mybir  # noqa: E402
from concourse._compat import with_exitstack  # noqa: E402
from concourse.bass_utils import run_bass_kernel_spmd  # noqa: E402

B, L, D, H, NS = 8, 1024, 1024, 16, 8
HD = D // H          # 64
LS = L - NS          # 1016
NCORES = 8
NEG = -1.0e9
BF = mybir.dt.bfloat16
F32 = mybir.dt.float32

_CACHE = {}
TRACE = False


@with_exitstack
def _attention_kernel(ctx: ExitStack, tc: tile.TileContext, aps: dict):
    nc = tc.nc

    sb = ctx.enter_context(tc.tile_pool(name="persist", bufs=1))
    wns_pool = ctx.enter_context(tc.tile_pool(name="wns", bufs=2))
    pt_pool = ctx.enter_context(tc.tile_pool(name="pt", bufs=2))
    stage = ctx.enter_context(tc.tile_pool(name="stage", bufs=2))
    ps = ctx.enter_context(tc.tile_pool(name="psum", bufs=1, space="PSUM"))
    psa = ctx.enter_context(tc.tile_pool(name="psacc", bufs=2, space="PSUM"))
    dram = ctx.enter_context(tc.tile_pool(name="dram", bufs=2, space="DRAM"))

    # ---- persistent SBUF tensors ----
    xT = sb.tile([128, 8 * 1024], BF)      # [d-part, dt*1024 + l]
    wq = sb.tile([128, 8 * 1024], BF)      # [d-part, dt*1024 + e]
    wk = sb.tile([128, 8 * 1024], BF)
    wv = sb.tile([128, 8 * 1024], BF)
    wo = sb.tile([128, 8 * 1024], BF)      # [e-part, et*1024 + e']
    qT = sb.tile([128, 8 * 1024], BF)      # [e-part, et*1024 + l]
    kT = sb.tile([128, 8 * 1024], BF)
    vb = sb.tile([128, 8 * 1040], BF)      # [l-part, lt*1040 + h*65 + eh]; col h*65+64 = ones
    oT = sb.tile([128, 8 * 1024], BF)      # [e-part, et*1024 + l]
    xtails = sb.tile([128, 64], BF)        # [d-part, dt*8 + bb]
    sel = sb.tile([64, 8], BF)             # one-hot row selector (per-core data)
    tri = sb.tile([128, 128], F32)         # tri[p,f] = 0 if p<=f else NEG
    mbias = sb.tile([128, 8], F32)         # key-padding additive bias per k-block
    ones1 = sb.tile([1, 128], BF)
    nsb = sb.tile([8, 3072], BF)           # my position's q|k|v for all batches
    fullg = sb.tile([64, 3072], BF)        # gathered: row n*8+bb

    # ---- input DMAs ----
    for dt in range(8):
        r = slice(dt * 128, dt * 128 + 128)
        nc.gpsimd.dma_start(xT[:, bass.ts(dt, 1024)], aps["xT"][r, :])
        nc.gpsimd.dma_start(wq[:, bass.ts(dt, 1024)], aps["wqT"][r, :])
        nc.gpsimd.dma_start(wk[:, bass.ts(dt, 1024)], aps["wkT"][r, :])
        nc.gpsimd.dma_start(wv[:, bass.ts(dt, 1024)], aps["wvT"][r, :])
        nc.gpsimd.dma_start(wo[:, bass.ts(dt, 1024)], aps["woutT"][r, :])
        nc.gpsimd.dma_start(xtails[:, bass.ts(dt, 8)], aps["xtails"][r, :])
    nc.gpsimd.dma_start(sel[:], aps["sel"][:])
    nc.gpsimd.dma_start(tri[:], aps["tri"][:])
    nc.gpsimd.dma_start(mbias[:], aps["maskbias"][:])
    nc.gpsimd.dma_start(ones1[:], aps["onesb"][:])

    # ---- phase 1: ns projections for my position (all batches) ----
    nsacc = sb.tile([8, 3072], F32)
    for dt in range(8):
        wt = wns_pool.tile([128, 3072], BF)
        nc.gpsimd.dma_start(wt[:], aps["wnsT"][dt * 128:dt * 128 + 128, :])
        for ck in range(6):
            pp = ps.tile([8, 512], F32, name="nsp", bufs=1)
            nc.tensor.matmul(
                pp[:],
                xtails[:, bass.ts(dt, 8)],
                wt[:, bass.ts(ck, 512)],
                start=True,
                stop=True,
            )
            if dt == 0:
                nc.vector.tensor_copy(nsacc[:, bass.ts(ck, 512)], pp[:])
            else:
                nc.vector.tensor_tensor(
                    nsacc[:, bass.ts(ck, 512)],
                    nsacc[:, bass.ts(ck, 512)],
                    pp[:],
                    mybir.AluOpType.add,
                )
    for ck in range(6):
        nc.vector.tensor_copy(nsb[:, bass.ts(ck, 512)], nsacc[:, bass.ts(ck, 512)])

    gin = dram.tile([8, 3072], BF)
    gout = dram.tile([64, 3072], BF)
    nc.gpsimd.dma_start(gin[:], nsb[:])
    nc.gpsimd.collective_compute(
        "AllGather",
        mybir.AluOpType.bypass,
        replica_groups=[list(range(NCORES))],
        ins=[gin.opt()],
        outs=[gout.opt()],
    )
    nc.gpsimd.dma_start(fullg[:], gout[:])

    # ---- phase 2: shared Q/K/V projections ----
    for w, dst in ((wq, qT), (wk, kT)):
        for et in range(8):
            for lg in range(2):
                acc = ps.tile([128, 512], F32, name="acc", bufs=2)
                for dt in range(8):
                    nc.tensor.matmul(
                        acc[:],
                        w[:, dt * 1024 + et * 128: dt * 1024 + et * 128 + 128],
                        xT[:, dt * 1024 + lg * 512: dt * 1024 + lg * 512 + 512],
                        start=(dt == 0),
                        stop=(dt == 7),
                    )
                nc.vector.tensor_copy(
                    dst[:, et * 1024 + lg * 512: et * 1024 + lg * 512 + 512], acc[:]
                )

    # V in natural [l, e] layout, interleaved with a ones column per head
    for lt in range(8):
        for hh in range(16):
            nc.vector.memset(vb[:, lt * 1040 + hh * 65 + 64: lt * 1040 + hh * 65 + 65], 1.0)
        for eg in range(2):
            acc = ps.tile([128, 512], F32, name="acc", bufs=2)
            for dt in range(8):
                nc.tensor.matmul(
                    acc[:],
                    xT[:, dt * 1024 + lt * 128: dt * 1024 + lt * 128 + 128],
                    wv[:, dt * 1024 + eg * 512: dt * 1024 + eg * 512 + 512],
                    start=(dt == 0),
                    stop=(dt == 7),
                )
            for hh in range(8):
                h = eg * 8 + hh
                nc.vector.tensor_copy(
                    vb[:, lt * 1040 + h * 65: lt * 1040 + h * 65 + 64],
                    acc[:, hh * 64: hh * 64 + 64],
                )

    # ---- phase 3: overwrite tail rows (1016..1023) from gathered ns results ----
    # q/k tails, transposed via selection matmul: lhsT=fullg chunk [64,128], rhs=sel [64,8]
    for c2 in range(16):
        tp = ps.tile([128, 512], F32, name="nsp", bufs=1)
        nc.tensor.matmul(
            tp[:, 0:8],
            fullg[:, c2 * 128: c2 * 128 + 128],
            sel[:],
            start=True,
            stop=True,
        )
        dst = qT if c2 < 8 else kT
        et = c2 % 8
        nc.vector.tensor_copy(dst[:, et * 1024 + 1016: et * 1024 + 1024], tp[:, 0:8])
    # v tails natural: lhsT=sel [64,8], rhs=fullg v cols
    for vg in range(2):
        tp = ps.tile([128, 512], F32, name="nsp", bufs=1)
        nc.tensor.matmul(
            tp[0:8, :],
            sel[:],
            fullg[:, 2048 + vg * 512: 2048 + vg * 512 + 512],
            start=True,
            stop=True,
        )
        vt = stage.tile([8, 512], BF, name="rec16")
        nc.vector.tensor_copy(vt[:], tp[0:8, :])
        for hh in range(8):
            h = vg * 8 + hh
            nc.gpsimd.dma_start(
                vb[120:128, 7 * 1040 + h * 65: 7 * 1040 + h * 65 + 64],
                vt[:, hh * 64: hh * 64 + 64],
            )

    # ---- phase 4: causal attention per head ----
    for h in range(H):
        et, r0 = h // 2, (h % 2) * 64
        for g in range(2):
            nj = 4 * g + 4
            pt = pt_pool.tile([128, 8 * 512], BF)
            for j in range(nj):
                lead = j * 128 - g * 512
                if lead > 0:
                    nc.vector.memset(pt[:, j * 512: j * 512 + lead], 0.0)
            for j in range(nj):
                sp = ps.tile([128, 512], F32, name="sp", bufs=2)
                nc.tensor.matmul(
                    sp[:],
                    kT[r0:r0 + 64, et * 1024 + j * 128: et * 1024 + j * 128 + 128],
                    qT[r0:r0 + 64, et * 1024 + g * 512: et * 1024 + g * 512 + 512],
                    start=True,
                    stop=True,
                )
                bias = mbias[:, j: j + 1]
                lead = max(0, j * 128 - g * 512)
                if j >= 4 * g:  # diagonal block lives in this q-group
                    nc.vector.tensor_tensor(
                        sp[:, lead: lead + 128],
                        sp[:, lead: lead + 128],
                        tri[:],
                        mybir.AluOpType.add,
                    )
                nc.scalar.activation(
                    pt[:, j * 512 + lead: j * 512 + 512],
                    sp[:, lead:512],
                    mybir.ActivationFunctionType.Exp,
                    bias=bias,
                    scale=0.125,
                )
            op = psa.tile([128, 512], F32)
            for j in range(nj):
                nc.tensor.matmul(
                    op[0:65, :],
                    vb[:, j * 1040 + h * 65: j * 1040 + h * 65 + 65],
                    pt[:, j * 512: j * 512 + 512],
                    start=(j == 0),
                    stop=(j == nj - 1),
                )
            rec32 = stage.tile([1, 512], F32)
            rec16 = stage.tile([1, 512], BF)
            nc.vector.reciprocal(rec32[:], op[64:65, :])
            nc.vector.tensor_copy(rec16[:], rec32[:])
            bp = ps.tile([128, 512], F32, name="bp", bufs=1)
            nc.tensor.matmul(bp[:], ones1[:], rec16[:], start=True, stop=True)
            bb = stage.tile([128, 512], F32, name="ys")
            nc.scalar.activation(
                bb[0:64, :], bp[0:64, :], mybir.ActivationFunctionType.Copy, scale=1.0
            )
            nc.vector.tensor_tensor(
                oT[r0:r0 + 64, et * 1024 + g * 512: et * 1024 + g * 512 + 512],
                op[0:64, :],
                bb[0:64, :],
                mybir.AluOpType.mult,
            )

    # ---- phase 5: output projection, natural [l, e'] ----
    for lt in range(8):
        for eg in range(2):
            acc = ps.tile([128, 512], F32, name="acc", bufs=2)
            for et in range(8):
                nc.tensor.matmul(
                    acc[:],
                    oT[:, et * 1024 + lt * 128: et * 1024 + lt * 128 + 128],
                    wo[:, et * 1024 + eg * 512: et * 1024 + eg * 512 + 512],
                    start=(et == 0),
                    stop=(et == 7),
                )
            ys = stage.tile([128, 512], F32)
            nc.vector.tensor_copy(ys[:], acc[:])
            nc.gpsimd.dma_start(
                aps["y"][lt * 128: lt * 128 + 128, eg * 512: eg * 512 + 512], ys[:]
            )


def _build():
    if "nc" in _CACHE:
        return _CACHE["nc"]
    nc = bacc.Bacc("TRN2", target_bir_lowering=False, debug=False, num_devices=NCORES)
    aps = {}
    for name, shape, dt in [
        ("xT", [1024, 1024], BF),
        ("wqT", [1024, 1024], BF),
        ("wkT", [1024, 1024], BF),
        ("wvT", [1024, 1024], BF),
        ("woutT", [1024, 1024], BF),
        ("wnsT", [1024, 3072], BF),
        ("xtails", [1024, 8], BF),
        ("sel", [64, 8], BF),
        ("tri", [128, 128], F32),
        ("maskbias", [128, 8], F32),
        ("onesb", [1, 128], BF),
    ]:
        aps[name] = nc.dram_tensor(name, shape, dt, kind="ExternalInput").ap()
    aps["y"] = nc.dram_tensor("y", [1024, 1024], F32, kind="ExternalOutput").ap()

    with tile.TileContext(nc) as tc:
        _attention_kernel(tc, aps)
    nc.compile()
    _CACHE["nc"] = nc
    return nc


def kernel(x, key_padding_mask, Wq_s, Wk_s, Wv_s, Wq_ns, Wk_ns, Wv_ns, W_out, **kw):
    x = np.asarray(x, np.float32)
    mask = np.asarray(key_padding_mask)
    bf = ml_dtypes.bfloat16

    wqT = np.ascontiguousarray(np.asarray(Wq_s, np.float32).T.astype(bf))
    wkT = np.ascontiguousarray(np.asarray(Wk_s, np.float32).T.astype(bf))
    wvT = np.ascontiguousarray(np.asarray(Wv_s, np.float32).T.astype(bf))
    woT = np.ascontiguousarray(np.asarray(W_out, np.float32).T.astype(bf))
    tri = np.where(
        np.arange(128)[:, None] <= np.arange(128)[None, :], 0.0, NEG
    ).astype(np.float32)
    onesb = np.ones((1, 128), bf)

    Wq_ns = np.asarray(Wq_ns, np.float32)
    Wk_ns = np.asarray(Wk_ns, np.float32)
    Wv_ns = np.asarray(Wv_ns, np.float32)

    in_maps = []
    for c in range(NCORES):
        xT = np.ascontiguousarray(x[c].T.astype(bf))
        xtails = np.ascontiguousarray(x[:, LS + c, :].T.astype(bf))  # [1024 d, 8 bb]
        wnsT = np.ascontiguousarray(
            np.concatenate([Wq_ns[c].T, Wk_ns[c].T, Wv_ns[c].T], axis=1).astype(bf)
        )
        selm = np.zeros((64, 8), bf)
        for n in range(NS):
            selm[n * 8 + c, n] = 1.0
        mb = np.where(mask[c], 0.0, NEG).astype(np.float32).reshape(8, 128).T
        mb = np.ascontiguousarray(mb)
        in_maps.append(
            {
                "xT": xT,
                "wqT": wqT,
                "wkT": wkT,
                "wvT": wvT,
                "woutT": woT,
                "wnsT": wnsT,
                "xtails": xtails,
                "sel": selm,
                "tri": tri,
                "maskbias": mb,
                "onesb": onesb,
            }
        )

    nc = _build()
    res = run_bass_kernel_spmd(nc, in_maps, list(range(NCORES)), trace=TRACE)
    _CACHE["exec_time_ns"] = res.exec_time_ns
    _CACHE["res"] = res
    out = np.stack([res.results[c]["y"] for c in range(NCORES)], axis=0)
    return out.astype(np.float32)



# revision 9
# speedup vs baseline: 1.3110x; 1.3110x over previous
"""Mixed causal attention (B=8,L=1024,D=1024,H=16,NS=8) on 8 TRN2 cores.

Sharding: data-parallel over batch (core b owns batch b) for projections,
attention, out-proj.  The per-position (ns) projection weights are sharded
by position: core c computes q/k/v for position 1016+c for ALL batches
(reads only Wq_ns[c],Wk_ns[c],Wv_ns[c]), then an AllGather distributes the
3x[8,1024] results; each core extracts its batch's 8 tail rows with a
one-hot selection matmul (the program is SPMD-identical, so per-core row
selection is driven by a per-core input, not program structure).

Attention layout: scores [k-part, q-free]; V tile carries a shared ones
column block per head pair so attn@V emits the numerator and a 64-way
replicated softmax denominator in one matmul; normalization is then one
fast approximate reciprocal + one multiply on the vector engine.
All score/exp/attn@V work is causally range-trimmed per k-block.
"""

import sys
import os
from contextlib import ExitStack

import numpy as np

sys.path.insert(0, "/opt/trn_rl_repo")

import ml_dtypes  # noqa: E402
import concourse.bass as bass  # noqa: E402
import concourse.tile as tile  # noqa: E402
from concourse import bacc, mybir  # noqa: E402
from concourse._compat import with_exitstack  # noqa: E402
from concourse.bass_utils import run_bass_kernel_spmd  # noqa: E402

B, L, D, H, NS = 8, 1024, 1024, 16, 8
HD = D // H          # 64
LS = L - NS          # 1016
NCORES = 8
NEG = -1.0e9
BF = mybir.dt.bfloat16
F32 = mybir.dt.float32

# vb layout: per lt block of 1536 cols, 8 head-pairs of 192 cols each:
# [v_{2p} (64) | ones (64) | v_{2p+1} (64)].  attn@V lhsT for even head
# = [v|ones] -> out rows 0:64 numerator, 64:128 denominator; odd head
# = [ones|v] -> out rows 0:64 denominator, 64:128 numerator.
VB_LT = 1536

_CACHE = {}
TRACE = False


@with_exitstack
def _attention_kernel(ctx: ExitStack, tc: tile.TileContext, aps: dict):
    nc = tc.nc

    sb = ctx.enter_context(tc.tile_pool(name="persist", bufs=1))
    wns_pool = ctx.enter_context(tc.tile_pool(name="wns", bufs=4))
    pt_pool = ctx.enter_context(tc.tile_pool(name="pt", bufs=8))
    stage = ctx.enter_context(tc.tile_pool(name="stage", bufs=2))
    dram = ctx.enter_context(tc.tile_pool(name="dram", bufs=2, space="DRAM"))

    # ---- persistent SBUF tensors ----
    xT = sb.tile([128, 8 * 1024], BF)      # [d-part, dt*1024 + l]
    wq = sb.tile([128, 8 * 1024], BF)      # [d-part, dt*1024 + e]
    wk = sb.tile([128, 8 * 1024], BF)
    wv = sb.tile([128, 8 * 1024], BF)
    wo = sb.tile([128, 8 * 1024], BF)      # [e-part, et*1024 + e']
    qT = sb.tile([128, 8 * 1024], BF)      # [e-part, et*1024 + l]
    kT = sb.tile([128, 8 * 1024], BF)
    vb = sb.tile([128, 8 * VB_LT], BF)     # [l-part, lt*1536 + pair*192 + ...]
    oT = sb.tile([128, 8 * 1024], BF)      # [e-part, et*1024 + l]
    xtails = sb.tile([128, 64], BF)        # [d-part, dt*8 + bb]
    sel = sb.tile([64, 8], BF)             # one-hot row selector (per-core data)
    tri = sb.tile([128, 128], F32)         # tri[p,f] = 0 if p<=f else NEG
    mbias = sb.tile([128, 8], F32)         # key-padding additive bias per k-block
    nsb = sb.tile([8, 3072], BF)           # my position's q|k|v for all batches
    fullg = sb.tile([64, 3072], BF)        # gathered: row n*8+bb

    # ---- input DMAs (gpsimd software DGE; consolidated) ----
    def chunked(ap):
        return ap.rearrange("(dt p) c -> p dt c", p=128)

    def chunked_dst(t, n=8):
        return t[:].rearrange("p (dt c) -> p dt c", dt=n)

    nc.gpsimd.dma_start(chunked_dst(xtails), chunked(aps["xtails"]))
    nc.gpsimd.dma_start(chunked_dst(xT), chunked(aps["xT"]))
    nc.gpsimd.dma_start(chunked_dst(wk), chunked(aps["wkT"]))
    nc.gpsimd.dma_start(tri[:], aps["tri"][:])
    nc.gpsimd.dma_start(mbias[:], aps["maskbias"][:])
    nc.gpsimd.dma_start(sel[:], aps["sel"][:])

    # ones columns of vb (one strided memset per lt; vector engine is idle now)
    for lt in range(8):
        blk = vb[:, lt * VB_LT: (lt + 1) * VB_LT]
        nc.vector.memset(
            blk.rearrange("p (pr x) -> p pr x", pr=8)[:, :, 64:128], 1.0
        )

    def proj_qk(w, dst):
        for et in range(8):
            for lg in range(2):
                acc = ps.tile([128, 512], F32, name="sp", bufs=4)
                for dt in range(8):
                    nc.tensor.matmul(
                        acc[:],
                        w[:, dt * 1024 + et * 128: dt * 1024 + et * 128 + 128],
                        xT[:, dt * 1024 + lg * 512: dt * 1024 + lg * 512 + 512],
                        start=(dt == 0),
                        stop=(dt == 7),
                    )
                nc.vector.tensor_copy(
                    dst[:, et * 1024 + lg * 512: et * 1024 + lg * 512 + 512], acc[:]
                )

    # ---- phase A: K projection (only needs xT + wk) ----
    with tc.tile_pool(name="psA", bufs=4, space="PSUM") as ps:
        proj_qk(wk, kT)

        # ---- phase B: ns projections for my position (all batches) ----
        # accumulate over dt in PSUM, 3 banks per half-pass (scoped pool)
        with tc.tile_pool(name="psNS", bufs=1, space="PSUM") as nsps:
            for half in range(2):
                pp = nsps.tile([8, 1536], F32)
                for dt in range(8):
                    wt = wns_pool.tile([128, 1536], BF)
                    nc.gpsimd.dma_start(
                        wt[:],
                        aps["wnsT"][
                            dt * 128: dt * 128 + 128,
                            half * 1536: half * 1536 + 1536,
                        ],
                    )
                    for ck in range(3):
                        nc.tensor.matmul(
                            pp[:, ck * 512: ck * 512 + 512],
                            xtails[:, bass.ts(dt, 8)],
                            wt[:, bass.ts(ck, 512)],
                            start=(dt == 0),
                            stop=(dt == 7),
                        )
                nc.vector.tensor_copy(
                    nsb[:, half * 1536: half * 1536 + 1536], pp[:]
                )

        gin = dram.tile([8, 3072], BF)
        gout = dram.tile([64, 3072], BF)
        nc.gpsimd.dma_start(gin[:], nsb[:])
        nc.gpsimd.collective_compute(
            "AllGather",
            mybir.AluOpType.bypass,
            replica_groups=[list(range(NCORES))],
            ins=[gin.opt()],
            outs=[gout.opt()],
        )

        # remaining weights (issued after the collective so the gather is not
        # delayed; transfers overlap the Q/V projections)
        nc.gpsimd.dma_start(chunked_dst(wq), chunked(aps["wqT"]))
        nc.gpsimd.dma_start(chunked_dst(wv), chunked(aps["wvT"]))
        nc.gpsimd.dma_start(chunked_dst(wo), chunked(aps["woutT"]))

        # ---- phase C: Q projection ----
        proj_qk(wq, qT)

        # ---- phase D: V projection, vb layout with shared ones blocks ----
        for lt in range(8):
            for eg in range(2):
                acc = ps.tile([128, 512], F32, name="sp", bufs=4)
                for dt in range(8):
                    nc.tensor.matmul(
                        acc[:],
                        xT[:, dt * 1024 + lt * 128: dt * 1024 + lt * 128 + 128],
                        wv[:, dt * 1024 + eg * 512: dt * 1024 + eg * 512 + 512],
                        start=(dt == 0),
                        stop=(dt == 7),
                    )
                # heads eg*8+hh -> pair (eg*4 + hh//2), parity hh%2
                dst = vb[:, lt * VB_LT + eg * 768: lt * VB_LT + eg * 768 + 768]
                dst3 = dst.rearrange("p (pr x) -> p pr x", pr=4)
                src3 = acc[:, :].rearrange("p (pr x) -> p pr x", pr=4)
                nc.vector.tensor_copy(dst3[:, :, 0:64], src3[:, :, 0:64])
                nc.vector.tensor_copy(dst3[:, :, 128:192], src3[:, :, 64:128])

        # fullg load must come after the weight DMAs on the gpsimd queue
        # (it blocks on the collective)
        nc.gpsimd.dma_start(fullg[:], gout[:])

        # ---- attention ----
        def attn_group(h, g, psa):
            et, r0 = h // 2, (h % 2) * 64
            nj = 4 * g + 4
            qbase = et * 1024 + g * 512
            pts = []
            for j in range(nj):
                lead = max(0, j * 128 - g * 512)
                sp = ps.tile([128, 512], F32, name="sp", bufs=4)
                nc.tensor.matmul(
                    sp[:, lead:512],
                    kT[r0:r0 + 64, et * 1024 + j * 128: et * 1024 + j * 128 + 128],
                    qT[r0:r0 + 64, qbase + lead: qbase + 512],
                    start=True,
                    stop=True,
                )
                if j >= 4 * g:  # leading q-sub-block is the diagonal block
                    nc.vector.tensor_tensor(
                        sp[:, lead: lead + 128],
                        sp[:, lead: lead + 128],
                        tri[:],
                        mybir.AluOpType.add,
                    )
                pt = pt_pool.tile([128, 512], BF)
                nc.scalar.activation(
                    pt[:, lead:512],
                    sp[:, lead:512],
                    mybir.ActivationFunctionType.Exp,
                    bias=mbias[:, j: j + 1],
                    scale=0.125,
                )
                pts.append((pt, lead))
            op = psa.tile([128, 512], F32)
            for j, (pt, lead) in enumerate(pts):
                vcol = j * VB_LT + (h // 2) * 192 + (h % 2) * 64
                nc.tensor.matmul(
                    op[:, lead:512],
                    vb[:, vcol: vcol + 128],
                    pt[:, lead:512],
                    start=(j == 0),
                    stop=(j == nj - 1),
                    skip_group_check=(j > 0),
                )
            nb = 64 * (h % 2)   # numerator base partition
            db = 64 - nb        # denominator base partition
            rb = stage.tile([64, 512], F32, name="rb")
            if db == 0:
                nc.vector.reciprocal_approx_fast(rb[:], op[0:64, :])
            else:
                # reciprocal_approx_fast requires base partition 0: stage den
                dcp = stage.tile([64, 512], F32, name="dcp")
                nc.vector.tensor_copy(dcp[:], op[64:128, :])
                nc.vector.reciprocal_approx_fast(rb[:], dcp[:])
            nc.vector.tensor_tensor(
                oT[r0:r0 + 64, qbase: qbase + 512],
                op[nb:nb + 64, :],
                rb[:],
                mybir.AluOpType.mult,
            )

        with tc.tile_pool(name="psB", bufs=3, space="PSUM") as psa:
            # g=0 q-groups have no dependence on the gathered tails
            for h in range(H):
                attn_group(h, 0, psa)

            # ---- tails: overwrite q/k rows 1016..1023, v rows from gather ----
            for c2 in range(16):
                tp = ps.tile([128, 512], F32, name="sp", bufs=4)
                nc.tensor.matmul(
                    tp[:, 0:8],
                    fullg[:, c2 * 128: c2 * 128 + 128],
                    sel[:],
                    start=True,
                    stop=True,
                )
                dst = qT if c2 < 8 else kT
                et = c2 % 8
                nc.vector.tensor_copy(
                    dst[:, et * 1024 + 1016: et * 1024 + 1024], tp[:, 0:8]
                )
            for vg in range(2):
                tp = ps.tile([128, 512], F32, name="sp", bufs=4)
                nc.tensor.matmul(
                    tp[0:8, :],
                    sel[:],
                    fullg[:, 2048 + vg * 512: 2048 + vg * 512 + 512],
                    start=True,
                    stop=True,
                )
                vt = stage.tile([8, 512], BF, name="vt")
                nc.vector.tensor_copy(vt[:], tp[0:8, :])
                # scatter into vb tail partitions 120..127, lt=7 slots
                base = 7 * VB_LT + vg * 768
                dst3 = vb[120:128, base: base + 768].rearrange(
                    "p (pr x) -> p pr x", pr=4
                )
                src3 = vt[:, :].rearrange("p (pr x) -> p pr x", pr=4)
                nc.gpsimd.dma_start(dst3[:, :, 0:64], src3[:, :, 0:64])
                nc.gpsimd.dma_start(dst3[:, :, 128:192], src3[:, :, 64:128])

            # ---- attention: g=1 q-groups ----
            for h in range(H):
                attn_group(h, 1, psa)

            # ---- output projection, natural [l, e'] ----
            for lt in range(8):
                for eg in range(2):
                    acc = ps.tile([128, 512], F32, name="sp", bufs=4)
                    for et in range(8):
                        nc.tensor.matmul(
                            acc[:],
                            oT[:, et * 1024 + lt * 128: et * 1024 + lt * 128 + 128],
                            wo[:, et * 1024 + eg * 512: et * 1024 + eg * 512 + 512],
                            start=(et == 0),
                            stop=(et == 7),
                        )
                    ys = stage.tile([128, 512], F32, name="ys")
                    nc.vector.tensor_copy(ys[:], acc[:])
                    nc.gpsimd.dma_start(
                        aps["y"][lt * 128: lt * 128 + 128, eg * 512: eg * 512 + 512],
                        ys[:],
                    )


def _build():
    if "nc" in _CACHE:
        return _CACHE["nc"]
    nc = bacc.Bacc("TRN2", target_bir_lowering=False, debug=False, num_devices=NCORES)
    aps = {}
    for name, shape, dt in [
        ("xT", [1024, 1024], BF),
        ("wqT", [1024, 1024], BF),
        ("wkT", [1024, 1024], BF),
        ("wvT", [1024, 1024], BF),
        ("woutT", [1024, 1024], BF),
        ("wnsT", [1024, 3072], BF),
        ("xtails", [1024, 8], BF),
        ("sel", [64, 8], BF),
        ("tri", [128, 128], F32),
        ("maskbias", [128, 8], F32),
    ]:
        aps[name] = nc.dram_tensor(name, shape, dt, kind="ExternalInput").ap()
    aps["y"] = nc.dram_tensor("y", [1024, 1024], F32, kind="ExternalOutput").ap()

    with tile.TileContext(nc) as tc:
        _attention_kernel(tc, aps)
    nc.compile()
    _CACHE["nc"] = nc
    return nc


def kernel(x, key_padding_mask, Wq_s, Wk_s, Wv_s, Wq_ns, Wk_ns, Wv_ns, W_out, **kw):
    x = np.asarray(x, np.float32)
    mask = np.asarray(key_padding_mask)
    bf = ml_dtypes.bfloat16

    wqT = np.ascontiguousarray(np.asarray(Wq_s, np.float32).T.astype(bf))
    wkT = np.ascontiguousarray(np.asarray(Wk_s, np.float32).T.astype(bf))
    wvT = np.ascontiguousarray(np.asarray(Wv_s, np.float32).T.astype(bf))
    woT = np.ascontiguousarray(np.asarray(W_out, np.float32).T.astype(bf))
    tri = np.where(
        np.arange(128)[:, None] <= np.arange(128)[None, :], 0.0, NEG
    ).astype(np.float32)

    Wq_ns = np.asarray(Wq_ns, np.float32)
    Wk_ns = np.asarray(Wk_ns, np.float32)
    Wv_ns = np.asarray(Wv_ns, np.float32)

    in_maps = []
    for c in range(NCORES):
        xT = np.ascontiguousarray(x[c].T.astype(bf))
        xtails = np.ascontiguousarray(x[:, LS + c, :].T.astype(bf))  # [1024 d, 8 bb]
        wnsT = np.ascontiguousarray(
            np.concatenate([Wq_ns[c].T, Wk_ns[c].T, Wv_ns[c].T], axis=1).astype(bf)
        )
        selm = np.zeros((64, 8), bf)
        for n in range(NS):
            selm[n * 8 + c, n] = 1.0
        mb = np.where(mask[c], 0.0, NEG).astype(np.float32).reshape(8, 128).T
        mb = np.ascontiguousarray(mb)
        in_maps.append(
            {
                "xT": xT,
                "wqT": wqT,
                "wkT": wkT,
                "wvT": wvT,
                "woutT": woT,
                "wnsT": wnsT,
                "xtails": xtails,
                "sel": selm,
                "tri": tri,
                "maskbias": mb,
            }
        )

    nc = _build()
    res = run_bass_kernel_spmd(nc, in_maps, list(range(NCORES)), trace=TRACE)
    _CACHE["exec_time_ns"] = res.exec_time_ns
    _CACHE["res"] = res
    out = np.stack([res.results[c]["y"] for c in range(NCORES)], axis=0)
    return out.astype(np.float32)


# revision 12
# speedup vs baseline: 1.5219x; 1.1609x over previous
"""Mixed causal attention (B=8,L=1024,D=1024,H=16,NS=8) on 8 TRN2 cores.

Sharding: data-parallel over batch (core b owns batch b) for projections,
attention, out-proj.  The per-position (ns) projection weights are sharded
by position: core c computes q/k/v for position 1016+c for ALL batches
(reads only Wq_ns[c],Wk_ns[c],Wv_ns[c]), then an AllGather distributes the
3x[8,1024] results; each core extracts its batch's 8 tail rows with a
one-hot selection matmul (the program is SPMD-identical, so per-core row
selection is driven by a per-core input, not program structure).

Attention layout: scores [k-part, q-free]; V tile carries a shared ones
column block per head pair so attn@V emits the numerator and a 64-way
replicated softmax denominator in one matmul; normalization is then one
fast approximate reciprocal + one multiply on the vector engine.
All score/exp/attn@V work is causally range-trimmed per k-block.
"""

import sys
import os
from contextlib import ExitStack

import numpy as np

sys.path.insert(0, "/opt/trn_rl_repo")

import ml_dtypes  # noqa: E402
import concourse.bass as bass  # noqa: E402
import concourse.tile as tile  # noqa: E402
from concourse import bacc, mybir  # noqa: E402
from concourse._compat import with_exitstack  # noqa: E402
from concourse.bass_utils import run_bass_kernel_spmd  # noqa: E402

B, L, D, H, NS = 8, 1024, 1024, 16, 8
HD = D // H          # 64
LS = L - NS          # 1016
NCORES = 8
NEG = -1.0e9
BF = mybir.dt.bfloat16
F32 = mybir.dt.float32

# vb layout: per lt block of 1536 cols, 8 head-pairs of 192 cols each:
# [v_{2p} (64) | ones (64) | v_{2p+1} (64)].  attn@V lhsT for even head
# = [v|ones] -> out rows 0:64 numerator, 64:128 denominator; odd head
# = [ones|v] -> out rows 0:64 denominator, 64:128 numerator.
VB_LT = 1536

_CACHE = {}
TRACE = False


@with_exitstack
def _attention_kernel(ctx: ExitStack, tc: tile.TileContext, aps: dict):
    nc = tc.nc

    sb = ctx.enter_context(tc.tile_pool(name="persist", bufs=1))
    wns_pool = ctx.enter_context(tc.tile_pool(name="wns", bufs=4))
    pt_pool = ctx.enter_context(tc.tile_pool(name="pt", bufs=12))
    stage = ctx.enter_context(tc.tile_pool(name="stage", bufs=2))
    dram = ctx.enter_context(tc.tile_pool(name="dram", bufs=2, space="DRAM"))

    # ---- persistent SBUF tensors ----
    xT = sb.tile([128, 8 * 1024], BF)      # [d-part, dt*1024 + l]
    wq = sb.tile([128, 8 * 1024], BF)      # [d-part, dt*1024 + e]
    wk = sb.tile([128, 8 * 1024], BF)
    wv = sb.tile([128, 8 * 1024], BF)
    wo = sb.tile([128, 8 * 1024], BF)      # [e-part, et*1024 + e']
    qT = sb.tile([128, 8 * 1024], BF)      # [e-part, et*1024 + l]
    kT = sb.tile([128, 8 * 1024], BF)
    vb = sb.tile([128, 8 * VB_LT], BF)     # [l-part, lt*1536 + pair*192 + ...]
    oT = sb.tile([128, 8 * 1024], BF)      # [e-part, et*1024 + l]
    xtails = sb.tile([128, 64], BF)        # [d-part, dt*8 + bb]
    sel = sb.tile([64, 8], BF)             # one-hot row selector (per-core data)
    tri = sb.tile([128, 128], F32)         # tri[p,f] = 0 if p<=f else NEG
    mbias = sb.tile([128, 8], F32)         # key-padding additive bias per k-block
    nsb = sb.tile([8, 3072], BF)           # my position's q|k|v for all batches
    fullg = sb.tile([64, 3072], BF)        # gathered: row n*8+bb

    # ---- input DMAs (gpsimd software DGE; consolidated) ----
    def chunked(ap):
        return ap.rearrange("(dt p) c -> p dt c", p=128)

    def chunked_dst(t, n=8):
        return t[:].rearrange("p (dt c) -> p dt c", dt=n)

    nc.gpsimd.dma_start(chunked_dst(xtails), chunked(aps["xtails"]))
    nc.gpsimd.dma_start(chunked_dst(xT)[:, 0:4, :], chunked(aps["xT"])[:, 0:4, :])
    nc.gpsimd.dma_start(chunked_dst(xT)[:, 4:8, :], chunked(aps["xT"])[:, 4:8, :])
    for et in range(8):
        nc.gpsimd.dma_start(
            chunked_dst(wk)[:, :, et * 128: et * 128 + 128],
            chunked(aps["wkT"])[:, :, et * 128: et * 128 + 128],
        )
    nc.gpsimd.dma_start(tri[:], aps["tri"][:])
    nc.gpsimd.dma_start(mbias[:], aps["maskbias"][:])
    nc.gpsimd.dma_start(sel[:], aps["sel"][:])

    # ones columns of vb (one strided memset per lt; vector engine is idle now)
    for lt in range(8):
        blk = vb[:, lt * VB_LT: (lt + 1) * VB_LT]
        nc.vector.memset(
            blk.rearrange("p (pr x) -> p pr x", pr=8)[:, :, 64:128], 1.0
        )

    def proj_qk(w, dst):
        for et in range(8):
            for lg in range(2):
                acc = ps.tile([128, 512], F32, name="sp", bufs=5)
                for dt in range(8):
                    nc.tensor.matmul(
                        acc[:],
                        w[:, dt * 1024 + et * 128: dt * 1024 + et * 128 + 128],
                        xT[:, dt * 1024 + lg * 512: dt * 1024 + lg * 512 + 512],
                        start=(dt == 0),
                        stop=(dt == 7),
                    )
                nc.vector.tensor_copy(
                    dst[:, et * 1024 + lg * 512: et * 1024 + lg * 512 + 512], acc[:]
                )

    # ---- phase A: K projection (only needs xT + wk) ----
    with tc.tile_pool(name="psA", bufs=4, space="PSUM") as ps:
        proj_qk(wk, kT)

        # ---- phase B: ns projections for my position (all batches) ----
        # accumulate over dt in PSUM, 3 banks per half-pass (scoped pool)
        with tc.tile_pool(name="psNS", bufs=1, space="PSUM") as nsps:
            for half in range(2):
                pp = nsps.tile([8, 1536], F32)
                for dt in range(8):
                    wt = wns_pool.tile([128, 1536], BF)
                    nc.gpsimd.dma_start(
                        wt[:],
                        aps["wnsT"][
                            dt * 128: dt * 128 + 128,
                            half * 1536: half * 1536 + 1536,
                        ],
                    )
                    for ck in range(3):
                        nc.tensor.matmul(
                            pp[:, ck * 512: ck * 512 + 512],
                            xtails[:, bass.ts(dt, 8)],
                            wt[:, bass.ts(ck, 512)],
                            start=(dt == 0),
                            stop=(dt == 7),
                        )
                nc.vector.tensor_copy(
                    nsb[:, half * 1536: half * 1536 + 1536], pp[:]
                )

        gin = dram.tile([8, 3072], BF)
        gout = dram.tile([64, 3072], BF)
        nc.gpsimd.dma_start(gin[:], nsb[:])
        nc.gpsimd.collective_compute(
            "AllGather",
            mybir.AluOpType.bypass,
            replica_groups=[list(range(NCORES))],
            ins=[gin.opt()],
            outs=[gout.opt()],
        )

        # remaining weights (issued after the collective so the gather is not
        # delayed; transfers overlap the Q/V projections)
        nc.gpsimd.dma_start(chunked_dst(wq), chunked(aps["wqT"]))
        nc.gpsimd.dma_start(chunked_dst(wv), chunked(aps["wvT"]))
        nc.gpsimd.dma_start(chunked_dst(wo), chunked(aps["woutT"]))

        # ---- phase C: Q projection ----
        proj_qk(wq, qT)

        # ---- phase D: V projection, vb layout with shared ones blocks ----
        for lt in range(8):
            for eg in range(2):
                acc = ps.tile([128, 512], F32, name="sp", bufs=5)
                for dt in range(8):
                    nc.tensor.matmul(
                        acc[:],
                        xT[:, dt * 1024 + lt * 128: dt * 1024 + lt * 128 + 128],
                        wv[:, dt * 1024 + eg * 512: dt * 1024 + eg * 512 + 512],
                        start=(dt == 0),
                        stop=(dt == 7),
                    )
                # heads eg*8+hh -> pair (eg*4 + hh//2), parity hh%2
                dst = vb[:, lt * VB_LT + eg * 768: lt * VB_LT + eg * 768 + 768]
                dst3 = dst.rearrange("p (pr x) -> p pr x", pr=4)
                src3 = acc[:, :].rearrange("p (pr x) -> p pr x", pr=4)
                nc.vector.tensor_copy(dst3[:, :, 0:64], src3[:, :, 0:64])
                nc.vector.tensor_copy(dst3[:, :, 128:192], src3[:, :, 64:128])

        # fullg load must come after the weight DMAs on the gpsimd queue
        # (it blocks on the collective)
        nc.gpsimd.dma_start(fullg[:], gout[:])

        # ---- attention ----
        use_bias = aps["use_bias"]

        def attn_scores(h, g):
            et, r0 = h // 2, (h % 2) * 64
            nj = 4 * g + 4
            qbase = et * 1024 + g * 512
            pts = []
            for j in range(nj):
                lead = max(0, j * 128 - g * 512)
                sp = ps.tile([128, 512], F32, name="sp", bufs=5)
                nc.tensor.matmul(
                    sp[:, lead:512],
                    kT[r0:r0 + 64, et * 1024 + j * 128: et * 1024 + j * 128 + 128],
                    qT[r0:r0 + 64, qbase + lead: qbase + 512],
                    start=True,
                    stop=True,
                )
                if j >= 4 * g:  # leading q-sub-block is the diagonal block
                    nc.vector.tensor_tensor(
                        sp[:, lead: lead + 128],
                        sp[:, lead: lead + 128],
                        tri[:],
                        mybir.AluOpType.add,
                    )
                pt = pt_pool.tile([128, 512], BF)
                kw = {"bias": mbias[:, j: j + 1]} if use_bias else {}
                nc.scalar.activation(
                    pt[:, lead:512],
                    sp[:, lead:512],
                    mybir.ActivationFunctionType.Exp,
                    scale=0.125,
                    **kw,
                )
                pts.append((pt, lead))
            return (h, g, pts)

        def attn_av(state, psa):
            h, g, pts = state
            et, r0 = h // 2, (h % 2) * 64
            nj = 4 * g + 4
            qbase = et * 1024 + g * 512
            op = psa.tile([128, 512], F32)
            for j, (pt, lead) in enumerate(pts):
                vcol = j * VB_LT + (h // 2) * 192 + (h % 2) * 64
                nc.tensor.matmul(
                    op[:, lead:512],
                    vb[:, vcol: vcol + 128],
                    pt[:, lead:512],
                    start=(j == 0),
                    stop=(j == nj - 1),
                    skip_group_check=(j > 0),
                )
            nb = 64 * (h % 2)   # numerator base partition
            db = 64 - nb        # denominator base partition
            rb = stage.tile([64, 512], F32, name="rb")
            if db == 0:
                nc.vector.reciprocal_approx_fast(rb[:], op[0:64, :])
            else:
                # reciprocal_approx_fast requires base partition 0: stage den
                dcp = stage.tile([64, 512], F32, name="dcp")
                nc.vector.tensor_copy(dcp[:], op[64:128, :])
                nc.vector.reciprocal_approx_fast(rb[:], dcp[:])
            nc.vector.tensor_tensor(
                oT[r0:r0 + 64, qbase: qbase + 512],
                op[nb:nb + 64, :],
                rb[:],
                mybir.AluOpType.mult,
            )

        def attn_phase(g, psa):
            # software-pipelined by one group: scores(i+1) issue before
            # attnV(i), so the exp chain has a full group of slack
            prev = None
            for h in range(H):
                st = attn_scores(h, g)
                if prev is not None:
                    attn_av(prev, psa)
                prev = st
            attn_av(prev, psa)

        with tc.tile_pool(name="psB", bufs=3, space="PSUM") as psa:
            # g=0 q-groups have no dependence on the gathered tails
            attn_phase(0, psa)

            # ---- tails: overwrite q/k rows 1016..1023, v rows from gather ----
            for c2 in range(16):
                tp = ps.tile([128, 512], F32, name="sp", bufs=5)
                nc.tensor.matmul(
                    tp[:, 0:8],
                    fullg[:, c2 * 128: c2 * 128 + 128],
                    sel[:],
                    start=True,
                    stop=True,
                )
                dst = qT if c2 < 8 else kT
                et = c2 % 8
                nc.vector.tensor_copy(
                    dst[:, et * 1024 + 1016: et * 1024 + 1024], tp[:, 0:8]
                )
            for vg in range(2):
                tp = ps.tile([128, 512], F32, name="sp", bufs=5)
                nc.tensor.matmul(
                    tp[0:8, :],
                    sel[:],
                    fullg[:, 2048 + vg * 512: 2048 + vg * 512 + 512],
                    start=True,
                    stop=True,
                )
                vt = stage.tile([8, 512], BF, name="vt")
                nc.vector.tensor_copy(vt[:], tp[0:8, :])
                # scatter into vb tail partitions 120..127, lt=7 slots
                base = 7 * VB_LT + vg * 768
                dst3 = vb[120:128, base: base + 768].rearrange(
                    "p (pr x) -> p pr x", pr=4
                )
                src3 = vt[:, :].rearrange("p (pr x) -> p pr x", pr=4)
                nc.gpsimd.dma_start(dst3[:, :, 0:64], src3[:, :, 0:64])
                nc.gpsimd.dma_start(dst3[:, :, 128:192], src3[:, :, 64:128])

            # ---- attention: g=1 q-groups ----
            attn_phase(1, psa)

            # ---- output projection, natural [l, e'] ----
            for lt in range(8):
                for eg in range(2):
                    acc = ps.tile([128, 512], F32, name="sp", bufs=5)
                    for et in range(8):
                        nc.tensor.matmul(
                            acc[:],
                            oT[:, et * 1024 + lt * 128: et * 1024 + lt * 128 + 128],
                            wo[:, et * 1024 + eg * 512: et * 1024 + eg * 512 + 512],
                            start=(et == 0),
                            stop=(et == 7),
                        )
                    ys = stage.tile([128, 512], F32, name="ys")
                    nc.vector.tensor_copy(ys[:], acc[:])
                    nc.gpsimd.dma_start(
                        aps["y"][lt * 128: lt * 128 + 128, eg * 512: eg * 512 + 512],
                        ys[:],
                    )


def _build(use_bias=True):
    key = ("nc", use_bias)
    if key in _CACHE:
        return _CACHE[key]
    nc = bacc.Bacc("TRN2", target_bir_lowering=False, debug=False, num_devices=NCORES)
    aps = {}
    for name, shape, dt in [
        ("xT", [1024, 1024], BF),
        ("wqT", [1024, 1024], BF),
        ("wkT", [1024, 1024], BF),
        ("wvT", [1024, 1024], BF),
        ("woutT", [1024, 1024], BF),
        ("wnsT", [1024, 3072], BF),
        ("xtails", [1024, 8], BF),
        ("sel", [64, 8], BF),
        ("tri", [128, 128], F32),
        ("maskbias", [128, 8], F32),
    ]:
        aps[name] = nc.dram_tensor(name, shape, dt, kind="ExternalInput").ap()
    aps["y"] = nc.dram_tensor("y", [1024, 1024], F32, kind="ExternalOutput").ap()
    aps["use_bias"] = use_bias

    with tile.TileContext(nc) as tc:
        _attention_kernel(tc, aps)
    nc.compile()
    _CACHE[key] = nc
    return nc


def kernel(x, key_padding_mask, Wq_s, Wk_s, Wv_s, Wq_ns, Wk_ns, Wv_ns, W_out, **kw):
    x = np.asarray(x, np.float32)
    mask = np.asarray(key_padding_mask)
    bf = ml_dtypes.bfloat16

    wqT = np.ascontiguousarray(np.asarray(Wq_s, np.float32).T.astype(bf))
    wkT = np.ascontiguousarray(np.asarray(Wk_s, np.float32).T.astype(bf))
    wvT = np.ascontiguousarray(np.asarray(Wv_s, np.float32).T.astype(bf))
    woT = np.ascontiguousarray(np.asarray(W_out, np.float32).T.astype(bf))
    tri = np.where(
        np.arange(128)[:, None] <= np.arange(128)[None, :], 0.0, NEG
    ).astype(np.float32)

    Wq_ns = np.asarray(Wq_ns, np.float32)
    Wk_ns = np.asarray(Wk_ns, np.float32)
    Wv_ns = np.asarray(Wv_ns, np.float32)

    in_maps = []
    for c in range(NCORES):
        xT = np.ascontiguousarray(x[c].T.astype(bf))
        xtails = np.ascontiguousarray(x[:, LS + c, :].T.astype(bf))  # [1024 d, 8 bb]
        wnsT = np.ascontiguousarray(
            np.concatenate([Wq_ns[c].T, Wk_ns[c].T, Wv_ns[c].T], axis=1).astype(bf)
        )
        selm = np.zeros((64, 8), bf)
        for n in range(NS):
            selm[n * 8 + c, n] = 1.0
        mb = np.where(mask[c], 0.0, NEG).astype(np.float32).reshape(8, 128).T
        mb = np.ascontiguousarray(mb)
        in_maps.append(
            {
                "xT": xT,
                "wqT": wqT,
                "wkT": wkT,
                "wvT": wvT,
                "woutT": woT,
                "wnsT": wnsT,
                "xtails": xtails,
                "sel": selm,
                "tri": tri,
                "maskbias": mb,
            }
        )

    nc = _build(use_bias=not bool(mask.all()))
    res = run_bass_kernel_spmd(nc, in_maps, list(range(NCORES)), trace=TRACE)
    _CACHE["exec_time_ns"] = res.exec_time_ns
    _CACHE["res"] = res
    out = np.stack([res.results[c]["y"] for c in range(NCORES)], axis=0)
    return out.astype(np.float32)
